# revision 16
# baseline (speedup 1.0000x reference)
"""Multi-head attention (B=4, S=2048, D=1024, H=16) on 8 Trainium2 cores.

Sharding: (batch, head-group) grid — core c handles batch c//2, heads
(c%2)*8..(c%2)*8+8. Zero duplicated FLOPs; host sums the two partial
out-projections per batch and adds bo.

Per-core kernel. Matmul operands in fp16 (1 cy/row like bf16, but 10-bit
mantissa; accumulation is always fp32 in PSUM), everything else fp32.

Layouts (all on-chip, no transposes anywhere):
  K^T/Q^T [512, 2048] feature-major; V [tok, head, 65] token-major with a
  ones column per head; scores computed k-major: S^T[k,q] = K.Q^T, so the
  exp output IS P^T, and the ones column makes the PV matmul accumulate the
  softmax denominator in PSUM row 64.

v2 changes (from trace analysis of the 540us baseline):
  * Window DMAs batched: one [128, 8, 512] descriptor per 512-token slice,
    shared by all 4 feature-tiles of a projection (HBM traffic 46MB -> 24MB,
    Sync-queue descriptor count 439 -> ~120).
  * Score matmuls emitted j-major: (j0,h0),(j0,h1),(j1,h0),(j1,h1). The two
    heads' K/Q live at base partitions 0/64 with K=64 contraction, so bass
    auto-derives tile_position (0,0)/(64,0) and the PE runs each pair
    CONCURRENTLY in disjoint row-group halves of the array (~2x on scores).
  * Exp split per k-tile (FD=512) so score banks free earlier, and 4/16 of
    exp tiles moved from the saturated ScalarE (was 92% busy) to VectorE
    via a Schraudolph fast-exp: round(s*1024*0.125/ln2 + (15360-57.77)) as
    int16 IS the fp16 bit pattern of exp(s/8) to within +-3.5% (zero mean);
    softmax division cancels the shared bias, end-to-end error ~1e-2 of max.
  * Softmax normalize muls moved to the idle GpSimd engine.
"""

import numpy as np

import bass_rust
import concourse.bass as bass
import concourse.tile as tile
from concourse import mybir

F32 = mybir.dt.float32
I16 = mybir.dt.int16
MMD = mybir.dt.float16     # matmul operand dtype

B, S, D = 4, 2048, 1024
NH, DK = 16, 64            # total heads, head dim
HG = 8                     # heads per core (head group)
DHG = HG * DK              # 512 features per head group
NP = 4                     # pairs of heads per core
QS = 512                   # q-slice size
NQS = S // QS              # 4
KT = S // 128              # 16 k-tiles
CT = D // 128              # 8 contraction chunks for projections
VW = DK + 1                # 65: V columns per head incl. ones column

# Schraudolph fast-exp (fp16): bitcast(round(x*FE_A + FE_B)) ~= exp(x/8)
FE_A = 1024.0 * 0.125 / float(np.log(2.0))
FE_B = 15360.0 - 57.77


def exp_on_dve(ktp, j, qsb):
    """Which exp tiles run on VectorE via fast-exp (avg 5/16 of all exp).

    Early q-slices run 4/16 (background projection work fills the PE),
    late ones 6/16 (no fill left -- ScalarE must stay off the critical
    path or the PE idles past the HAM window and the clock drops)."""
    if qsb < 2:
        return j == 1 and ktp in (1, 3, 5, 7)
    return (j == 1 and ktp in (1, 3, 5, 7)) or (j == 0 and ktp in (2, 6))


def split_multi_waits(nc):
    """This toolchain's walrus accepts only ONE sync-wait per instruction;
    Tile attaches several (one per producer proc). Hoist all but one wait
    onto single-wait NOPs inserted just before the instruction on the same
    engine (engines are in-order, so semantics are identical)."""
    uid = 0
    for f in nc.m.functions:
        for bb in f.blocks:
            il = bb.instructions
            i = 0
            while i < len(il):
                inst = il[i]
                si = inst.sync_info
                if si is not None and len(si.on_wait) > 1:
                    waits = list(si.on_wait)
                    inst.sync_info = bass_rust.SyncInfo(
                        on_wait=[waits[-1]], on_update=list(si.on_update)
                    )
                    for w in waits[:-1]:
                        nop = mybir.InstNoOp(
                            name=f"WSPLIT-{uid}",
                            engine=inst.engine,
                            bass_nofuse=True,
                            sync_info=bass_rust.SyncInfo(
                                on_wait=[w], on_update=[]
                            ),
                        )
                        uid += 1
                        il.insert(i, nop)
                        i += 1
                i += 1


def bcast_ap(ap, parts, n):
    """Partition-broadcast view of a DRAM row AP: [[0,parts],[1,n]]."""
    return bass.AP(tensor=ap.tensor, offset=ap.offset, ap=[[0, parts], [1, n]])


def build_kernel():
    nc = bass.Bass(trn_type="TRN2")

    xq = nc.dram_tensor("xq", (D, S), MMD, kind="ExternalInput")   # query[b].T
    xk = nc.dram_tensor("xk", (D, S), MMD, kind="ExternalInput")
    xv = nc.dram_tensor("xv", (D, S), MMD, kind="ExternalInput")
    wq = nc.dram_tensor("wq", (D, DHG), MMD, kind="ExternalInput")  # Wq[hg].T
    wk = nc.dram_tensor("wk", (D, DHG), MMD, kind="ExternalInput")
    wv = nc.dram_tensor("wv", (D, DHG), MMD, kind="ExternalInput")
    wo = nc.dram_tensor("wo", (DHG, D), MMD, kind="ExternalInput")  # Wo[:,hg].T
    bq = nc.dram_tensor("bq", (DHG,), F32, kind="ExternalInput")
    bk = nc.dram_tensor("bk", (DHG,), F32, kind="ExternalInput")
    bv = nc.dram_tensor("bv", (DHG,), F32, kind="ExternalInput")
    out = nc.dram_tensor("out", (S, D), F32, kind="ExternalOutput")

    from contextlib import ExitStack

    with tile.TileContext(nc) as tc, ExitStack() as ctx:
        persist = ctx.enter_context(tc.tile_pool(name="persist", bufs=1))
        KT_sb = persist.tile([128, NP, S], MMD)        # K^T: pair p rows
        QT_sb = persist.tile([128, NP, S], MMD)        # Q^T
        V_sb = persist.tile([128, KT, HG, VW], MMD)    # V token-major + ones
        AON = persist.tile([128, NP, S], MMD)          # normalized AO^T
        wk_sb = persist.tile([128, CT, DHG], MMD)
        wq_sb = persist.tile([128, CT, DHG], MMD)
        wv_sb = persist.tile([128, CT, DHG], MMD)
        wo_sb = persist.tile([128, NP, D], MMD)
        bq_sb = persist.tile([128, NP], F32)
        bk_sb = persist.tile([128, NP], F32)
        bv_bc = persist.tile([128, DHG], F32)

        # wk + biases first so the first K-projection matmuls start ~6us in;
        # the x windows are interleaved with the remaining weight loads in
        # the emission schedule below (all on the one sync DMA queue)
        nc.sync.dma_start(wk_sb[:], wk.rearrange("(c p) n -> p c n", p=128))
        with nc.allow_non_contiguous_dma(reason="tiny bias loads"):
            nc.sync.dma_start(bq_sb[:], bq.rearrange("(t p) -> p t", p=128))
            nc.sync.dma_start(bk_sb[:], bk.rearrange("(t p) -> p t", p=128))
        nc.vector.memset(V_sb[:, :, :, DK], 1.0)       # ones columns

        pmm = ctx.enter_context(tc.tile_pool(name="pmm", bufs=1, space="PSUM"))
        xpool = ctx.enter_context(tc.tile_pool(name="xw", bufs=8))
        ptp = ctx.enter_context(tc.tile_pool(name="ptile", bufs=5))
        npool = ctx.enter_context(tc.tile_pool(name="norm", bufs=3))
        opool = ctx.enter_context(tc.tile_pool(name="ostage", bufs=2))
        dpool = ctx.enter_context(
            tc.tile_pool(name="dscratch", bufs=3, space="DRAM")
        )

        def window(xdram, qs):
            """One DMA for all 8 contraction chunks of a 512-token slice."""
            xc = xpool.tile([128, CT, QS], MMD, tag="xw", name="xw")
            nc.sync.dma_start(
                xc[:],
                xdram.rearrange("(c p) n -> p c n", p=128)[
                    :, :, qs * QS:(qs + 1) * QS
                ],
            )
            return xc

        def kq_jt(win, w_sb, dst, b_sb, jt, qs):
            """dst[:, jt, qs] = w[:, :, jt].T @ x^T[:, qs] + bias."""
            ps = pmm.tile([128, QS], F32, tag="pj", name="pj", bufs=2)
            for ct in range(CT):
                nc.tensor.matmul(
                    ps[:],
                    w_sb[:, ct, jt * 128:(jt + 1) * 128],
                    win[:, ct, :],
                    start=(ct == 0), stop=(ct == CT - 1),
                )
            nc.vector.tensor_scalar_add(
                dst[:, jt, qs * QS:(qs + 1) * QS], ps[:], b_sb[:, jt:jt + 1]
            )

        def v_tiles(qs):
            """V_sb tok-tiles for one 512-token slice (4 tiles)."""
            win = window(xv, qs)
            for i in range(4):
                tt = qs * 4 + i
                ps = pmm.tile([128, DHG], F32, tag="pj", name="pjv", bufs=2)
                for ct in range(CT):
                    nc.tensor.matmul(
                        ps[:],
                        win[:, ct, i * 128:(i + 1) * 128],
                        wv_sb[:, ct, :],
                        start=(ct == 0), stop=(ct == CT - 1),
                    )
                nc.vector.tensor_add(
                    V_sb[:, tt, :, 0:DK],
                    ps[:].rearrange("p (h d) -> p h d", d=DK),
                    bv_bc[:].rearrange("p (h d) -> p h d", d=DK),
                )

        def attention(p, qsb):
            """One head-pair over one 512-wide q-slice.

            The whole inner loop runs in the PE's 64x128 row-tiled mode
            (tiles T0 = SBUF partitions 0-63, T8 = 64-127): the two heads'
            score matmuls (K=64, base partitions 0/64) and the row-split AV
            halves land on alternating tiles, so back-to-back instructions
            execute concurrently and LDWEIGHTS stays hidden. st/pt tiles are
            paired by k-tile j (h2 inner) so ONE exp instruction gates both
            heads' next step: their score MMs become ready together and the
            scheduler keeps the concurrent pairs adjacent.
            """
            q0 = qsb * QS
            ao = [
                pmm.tile([VW, QS], F32, tag=f"ao{h2}", name=f"ao{h2}")
                for h2 in range(2)
            ]
            for ktp in range(KT // 2):
                st = [
                    pmm.tile([128, 2, QS], F32, tag="st", name=f"st{j}",
                             bufs=2)
                    for j in range(2)
                ]
                for j in range(2):
                    kt = 2 * ktp + j
                    for h2 in range(2):
                        lo, hi = h2 * DK, h2 * DK + DK
                        nc.tensor.matmul(
                            st[j][:, h2, :],
                            KT_sb[lo:hi, p, kt * 128:(kt + 1) * 128],
                            QT_sb[lo:hi, p, q0:q0 + QS],
                            start=True, stop=True,
                        )
                pt = [
                    ptp.tile([128, 2, QS], MMD, tag="pt", name=f"pt{j}")
                    for j in range(2)
                ]
                for j in range(2):
                    if exp_on_dve(ktp, j, qsb):
                        nc.vector.tensor_scalar(
                            pt[j][:].bitcast(I16),
                            st[j][:],
                            FE_A, FE_B,
                            mybir.AluOpType.mult, mybir.AluOpType.add,
                        )
                    else:
                        nc.scalar.activation(
                            pt[j][:], st[j][:],
                            mybir.ActivationFunctionType.Exp,
                            scale=0.125,
                        )
                for j in range(2):
                    kt = 2 * ktp + j
                    for h2 in range(2):
                        nc.tensor.matmul(
                            ao[h2][:],
                            V_sb[:, kt, 2 * p + h2, :],
                            pt[j][:, h2, :],
                            start=(kt == 0), stop=(kt == KT - 1),
                        )
            for h2 in range(2):
                # copy to SBUF promptly so PSUM frees fast
                aos = npool.tile([VW, QS], F32, tag="aos", name="aos")
                nc.vector.tensor_copy(aos[:], ao[h2][:])
                # full-lane reciprocal via [1,512] -> [128,4] DRAM reshape
                dn = dpool.tile([1, QS], F32, tag="dn", name="dn")
                nc.sync.dma_start(dn[:], aos[DK:VW, :])
                rc = npool.tile([128, 4], F32, tag="rc", name="rc")
                nc.sync.dma_start(
                    rc[:], dn[:].rearrange("x (p j) -> (x p) j", j=4)
                )
                nc.vector.reciprocal(rc[:], rc[:])
                rcd = dpool.tile([1, QS], F32, tag="rcd", name="rcd")
                nc.sync.dma_start(
                    rcd[:].rearrange("x (p j) -> (x p) j", j=4), rc[:]
                )
                rb = npool.tile([DK, QS], F32, tag="rb", name="rb")
                nc.sync.dma_start(rb[:], bcast_ap(rcd[:], DK, QS))
                # gpsimd is idle so it takes the normalize muls; the final
                # pair is latency-critical (tail) -> faster on VectorE
                eng = nc.vector if (qsb == NQS - 1 and p == NP - 1) else \
                    nc.gpsimd
                eng.tensor_mul(
                    AON[h2 * DK:(h2 + 1) * DK, p, q0:q0 + QS],
                    aos[0:DK, :],
                    rb[:],
                )

        def outproj_tile(qsb, tt):
            """Out-projection for token tile tt (128 rows) of q-slice qsb."""
            q0 = qsb * QS
            ot = opool.tile([128, D], F32, tag="ot", name="ot")
            for oh in range(2):
                po = pmm.tile([128, 512], F32, tag="pj", name="po", bufs=2)
                for ci in range(NP):
                    nc.tensor.matmul(
                        po[:],
                        AON[:, ci, q0 + tt * 128:q0 + (tt + 1) * 128],
                        wo_sb[:, ci, oh * 512:(oh + 1) * 512],
                        start=(ci == 0), stop=(ci == NP - 1),
                    )
                # early tiles evacuate on ScalarE (it has slack while proj
                # fill runs); late ones stay on VectorE to keep ScalarE
                # exp-only when it is the pacing engine
                if qsb < 2:
                    nc.scalar.copy(ot[:, oh * 512:(oh + 1) * 512], po[:])
                else:
                    nc.vector.tensor_copy(
                        ot[:, oh * 512:(oh + 1) * 512], po[:]
                    )
            nc.sync.dma_start(out[q0 + tt * 128:q0 + (tt + 1) * 128, :], ot[:])

        # ---- emission schedule ---------------------------------------------
        # Normal priority: K proj (all pairs, windows shared across pairs),
        # Q pair-0 slice-0, V, then the attention stream + out-projections.
        # Remaining Q projections are demoted to background priority: the
        # scheduler pulls them early only when a data dependency demands it,
        # and otherwise uses them to fill PE idle slots -- keeping the PE
        # dense and the clock warm.
        kwins = [window(xk, qs) for qs in range(NQS)]
        qwin0 = window(xq, 0)
        nc.sync.dma_start(wq_sb[:], wq.rearrange("(c p) n -> p c n", p=128))
        nc.sync.dma_start(wv_sb[:], wv.rearrange("(c p) n -> p c n", p=128))
        nc.sync.dma_start(bv_bc[:], bcast_ap(bv[:], 128, DHG))
        nc.sync.dma_start(wo_sb[:], wo.rearrange("(c p) n -> p c n", p=128))
        for qs in range(NQS):
            kq_jt(kwins[qs], wk_sb, KT_sb, bk_sb, 0, qs)
        kq_jt(qwin0, wq_sb, QT_sb, bq_sb, 0, 0)
        for qs in range(NQS):
            v_tiles(qs)
        for jt in range(1, NP):
            for qs in range(NQS):
                kq_jt(kwins[qs], wk_sb, KT_sb, bk_sb, jt, qs)

        with tc.high_priority(offset=-(10 ** 6)):
            for jt in range(1, NP):
                kq_jt(qwin0, wq_sb, QT_sb, bq_sb, jt, 0)
            for qs in range(1, NQS):
                qwin = window(xq, qs)
                for jt in range(NP):
                    kq_jt(qwin, wq_sb, QT_sb, bq_sb, jt, qs)

        for qsb in range(NQS):
            for p in range(NP):
                attention(p, qsb)
                if qsb > 0:
                    outproj_tile(qsb - 1, p)
        for tt in range(NQS):
            outproj_tile(NQS - 1, tt)

    split_multi_waits(nc)
    return nc


def _prep_inputs(query, key, value, Wq, bq, Wk, bk, Wv, bv, Wo, bo):
    """Build the 8 per-core input maps."""
    def cvt(a):
        return np.ascontiguousarray(a.astype(np.float16))

    xt = {}
    for nm, x in (("xq", query), ("xk", key), ("xv", value)):
        xt[nm] = [cvt(x[b].T) for b in range(B)]
    in_maps = []
    for c in range(8):
        b, g = divmod(c, 2)
        rows = slice(g * DHG, (g + 1) * DHG)
        in_maps.append({
            "xq": xt["xq"][b], "xk": xt["xk"][b], "xv": xt["xv"][b],
            "wq": cvt(Wq[rows, :].T),
            "wk": cvt(Wk[rows, :].T),
            "wv": cvt(Wv[rows, :].T),
            "wo": cvt(Wo[:, rows].T),
            "bq": np.ascontiguousarray(bq[rows]),
            "bk": np.ascontiguousarray(bk[rows]),
            "bv": np.ascontiguousarray(bv[rows]),
        })
    return in_maps


_NC_CACHE = None


def run(inputs, trace=False):
    """Returns (full_output, BassKernelResults)."""
    global _NC_CACHE
    from concourse.bass_utils import run_bass_kernel_spmd

    inputs = {k: np.asarray(v, np.float32) for k, v in inputs.items()}
    in_maps = _prep_inputs(**inputs)
    if _NC_CACHE is None:
        _NC_CACHE = build_kernel()
    res = run_bass_kernel_spmd(
        _NC_CACHE, in_maps, core_ids=list(range(8)), trace=trace
    )
    bo = inputs["bo"]
    full = np.empty((B, S, D), np.float32)
    for b in range(B):
        full[b] = res.results[2 * b]["out"] + res.results[2 * b + 1]["out"] + bo
    return full, res


def kernel(**inputs):
    return run(inputs, trace=False)[0]
